# revision 4
# baseline (speedup 1.0000x reference)
"""Depthwise cross-correlation (SiamFC-style) Trainium2 kernel, v2.

z: [128, 256, 7, 7] templates, x: [128, 256, 31, 31] search images.
out[b,c,p,q] = sum_{i,j} z[b,c,i,j] * x[b,c,p+i,q+j]  -> [128, 256, 25, 25]

Pure data parallel over batch (16 batches/core on 8 cores); per core
4096 (b,c) pairs = 32 tiles of 128 partitions.

v2 engine plan (per tile), from microbenchmarked costs:
- DMA: x loaded CONTIGUOUS [128, 961] split across 2 queues (strided
  31x124B loads cost 7.7us vs 2x ~2.3us contiguous halves).
- ACT: re-pitch copy 31->32 cols (f32, ~1us) so fp32r moving APs have
  even strides; plus a few diag weight builds.
- PE: K_PE taps as fp32r full-width diag-matmul interleaved accumulation
  chains (measured 80ns per N=338 matmul: fp32r streams 2 cols/cycle).
- DVE: remaining taps as fused scalar_tensor_tensor MACs round-robin
  over NACC independent accumulators (independent ops pipeline ~5x
  better than a dependent chain), then merge tree + PSUM merges.
- GPSIMD: most diag weight builds (SBUF-only engine, otherwise idle).
"""

import numpy as np

B, C = 128, 256
HZ, WZ = 7, 7
HX, WX = 31, 31
HO, WO = 25, 25
N_CORES = 8
B_PER_CORE = B // N_CORES            # 16
PAIRS = B_PER_CORE * C               # 4096 channel pairs per core
NTILES = PAIRS // 128                # 32
XF = HX * WX                         # 961
ZF = HZ * WZ                         # 49
OF = HO * WO                         # 625
WQ = 26                              # padded q-window (col 25 discarded)
WXP = 32                             # padded x row pitch
P_SPLIT = 13                         # psum chunk A rows; B = 12
NA = P_SPLIT * WQ                    # 338
NB = (HO - P_SPLIT) * WQ             # 312

K_PE = 32                            # taps on PE (diag matmul chains)
NACC = 4                             # DVE accumulators (indep pipelining)
N_BUILD_ACT = 6                      # weight builds on ACT (rest gpsimd)
XSPLIT = 480                         # x DMA split point (2 queues)


def _install_tilefix():
    """This walrus build accepts only one sync-wait command on a Drain.
    Split the TileContext tail-drain waits across single-wait SP nops."""
    import concourse.tile as tile_mod
    from concourse.vector_clock import ScopedClock

    def _drain_and_barrier_split(self, tick_clock, wait_clock):
        nc = self.nc
        probe = nc.sync.nop(nofuse=True, hint="drain_wait_probe")
        wait_clock.add_sem_waits(
            probe.ins, ScopedClock({None: tick_clock.global_clock})
        )
        si = probe.ins.sync_info
        waits = list(si.on_wait) if si is not None and si.on_wait else []
        if si is not None:
            si.on_wait = waits[:1]
        for w in waits[1:]:
            stub = nc.sync.nop(nofuse=True, hint="drain_wait_split")
            ssi = stub.ins.sync_info
            if ssi is None:
                import concourse.mybir as mybir
                stub.ins.sync_info = mybir.SyncInfo(on_wait=[w], on_update=[])
            else:
                ssi.on_wait = list(ssi.on_wait or []) + [w]
        nc.sync.drain()
        nc.all_engine_barrier()
        assert self.sems is not None
        popped = nc._tile_sem_poison_stack.pop()
        assert popped is self._sem_poison
        nc.clear_and_free_semaphores(list(self.sems.allocated().values()))
        nc.all_engine_barrier()

    tile_mod.TileContext._drain_and_barrier = _drain_and_barrier_split


def _split_multi_waits(nc):
    """This walrus build accepts only one sync-wait command per instruction.
    Hoist extra waits onto single-wait nops on the same engine just before."""
    import concourse.mybir as mybir

    n = 0
    for f in nc.m.functions:
        for bb in f.blocks:
            insts = list(bb.instructions)
            out_insts = []
            changed = False
            for inst in insts:
                si = inst.sync_info
                if si is not None and si.on_wait and len(si.on_wait) > 1:
                    waits = list(si.on_wait)
                    si.on_wait = waits[-1:]
                    for w in waits[:-1]:
                        n += 1
                        out_insts.append(mybir.InstNoOp(
                            name=f"waitsplit-{n}",
                            engine=inst.engine,
                            bass_nofuse=True,
                            sync_info=mybir.SyncInfo(on_wait=[w], on_update=[]),
                        ))
                    changed = True
                out_insts.append(inst)
            if changed:
                bb.instructions.clear()
                for inst in out_insts:
                    bb.add_instruction(inst)
    return n


_NC_CACHE = {}


def _build_bass(reps: int = 1, timing: bool = False):
    import concourse.bass as bass
    import concourse.mybir as mybir
    import concourse.tile as tile
    from concourse.masks import make_identity

    _install_tilefix()

    f32 = mybir.dt.float32
    f32r = mybir.dt.float32r

    nc = bass.Bass()
    xs = nc.declare_dram_parameter("xs", [PAIRS, XF], f32, isOutput=False)
    zs = nc.declare_dram_parameter("zs", [PAIRS, ZF], f32, isOutput=False)
    out_rows = 128 if timing else PAIRS
    out = nc.declare_dram_parameter("out", [out_rows, OF], f32, isOutput=True)

    pe_taps = list(range(K_PE))
    dve_taps = list(range(K_PE, ZF))

    with tile.TileContext(nc) as tc:
        with (
            tc.tile_pool(name="consts", bufs=1) as consts,
            tc.tile_pool(name="xcin", bufs=3) as xcin,
            tc.tile_pool(name="xin", bufs=3) as xin,
            tc.tile_pool(name="zin", bufs=3) as zin,
            tc.tile_pool(name="wts", bufs=2) as wts,
            tc.tile_pool(name="accp", bufs=2) as accp,
            tc.tile_pool(name="outp", bufs=3) as outp,
            tc.tile_pool(name="psum", bufs=3, space="PSUM") as psum,
        ):
            ident = consts.tile([128, 128], f32)
            make_identity(nc, ident)

            def win(x_t, i, j, p0, pc, wq):
                return x_t[:, i + p0:i + p0 + pc, j:j + wq]

            for _rep in range(reps):
              for t in range(NTILES):
                r0 = t * 128
                # contiguous x load, split across two DMA queues
                xc = xcin.tile([128, XF], f32)
                nc.sync.dma_start(out=xc[:, 0:XSPLIT],
                                  in_=xs[r0:r0 + 128, 0:XSPLIT])
                nc.gpsimd.dma_start(out=xc[:, XSPLIT:XF],
                                    in_=xs[r0:r0 + 128, XSPLIT:XF])
                z_t = zin.tile([128, ZF], f32)
                nc.sync.dma_start(out=z_t, in_=zs[r0:r0 + 128, :])

                # re-pitch 31->32 cols on ACT; zero the pad column once
                x_t = xin.tile([128, HX, WXP], f32r)
                x_f = x_t.bitcast(f32)
                nc.scalar.copy(
                    x_t[:, :, 0:WX],
                    xc.rearrange("p (h w) -> p h w", h=HX))
                nc.vector.tensor_scalar_mul(
                    x_t[:, :, WX], xc[:, 0:HX], 0.0)

                # diag weights for PE taps (gpsimd + a few on ACT)
                w_all = wts.tile([128, K_PE, 128], f32r)
                for n, tap in enumerate(pe_taps):
                    zcol = z_t[:, tap:tap + 1]
                    if n < N_BUILD_ACT:
                        nc.scalar.mul(w_all[:, n, :], ident, zcol)
                    else:
                        nc.gpsimd.tensor_scalar_mul(w_all[:, n, :], ident, zcol)

                # PE: interleaved two-bank fp32r accumulation chains
                ps_a = psum.tile([128, P_SPLIT, WQ], f32)
                ps_b = psum.tile([128, HO - P_SPLIT, WQ], f32)
                n_pe = len(pe_taps)
                for n, tap in enumerate(pe_taps):
                    i, j = divmod(tap, WZ)
                    w_n = w_all[:, n, :]
                    nc.tensor.matmul(
                        ps_a, w_n, win(x_t, i, j, 0, P_SPLIT, WQ),
                        start=(n == 0), stop=(n == n_pe - 1),
                        skip_group_check=True)
                    nc.tensor.matmul(
                        ps_b, w_n, win(x_t, i, j, P_SPLIT, HO - P_SPLIT, WQ),
                        start=(n == 0), stop=(n == n_pe - 1),
                        skip_group_check=True)

                # DVE: fused MAC taps, round-robin over NACC accumulators
                acc = accp.tile([128, NACC, HO, WO], f32)
                for n, tap in enumerate(dve_taps):
                    i, j = divmod(tap, WZ)
                    k = n % NACC
                    x_win = win(x_f, i, j, 0, HO, WO)
                    zcol = z_t[:, tap:tap + 1]
                    if n < NACC:
                        nc.vector.tensor_scalar_mul(acc[:, k], x_win, zcol)
                    else:
                        nc.vector.scalar_tensor_tensor(
                            acc[:, k], x_win, zcol, acc[:, k],
                            op0=mybir.AluOpType.mult,
                            op1=mybir.AluOpType.add)

                # merge: acc tree + psum chunks -> output staging
                m01 = acc[:, 0]
                m23 = acc[:, 2]
                nc.vector.tensor_add(m01, acc[:, 0], acc[:, 1])
                nc.vector.tensor_add(m23, acc[:, 2], acc[:, 3])
                nc.vector.tensor_add(m01, m01, m23)
                o_t = outp.tile([128, HO, WO], f32)
                nc.vector.tensor_add(
                    o_t[:, 0:P_SPLIT, :], m01[:, 0:P_SPLIT, :],
                    ps_a[:, :, 0:WO])
                nc.vector.tensor_add(
                    o_t[:, P_SPLIT:HO, :], m01[:, P_SPLIT:HO, :],
                    ps_b[:, :, 0:WO])

                o0 = 0 if timing else r0
                o_flat = o_t.rearrange("p h w -> p (h w)")
                nc.sync.dma_start(out=out[o0:o0 + 128, 0:NA],
                                  in_=o_flat[:, 0:NA])
                nc.scalar.dma_start(out=out[o0:o0 + 128, NA:OF],
                                    in_=o_flat[:, NA:OF])

    _split_multi_waits(nc)
    return nc


def _get_nc(reps: int = 1, timing: bool = False):
    key = ("nc", reps, timing)
    if key not in _NC_CACHE:
        _NC_CACHE[key] = _build_bass(reps, timing)
    return _NC_CACHE[key]


def kernel(z: np.ndarray, x: np.ndarray, _trace: bool = False):
    from concourse.bass_utils import run_bass_kernel_spmd

    z = np.ascontiguousarray(z, dtype=np.float32)
    x = np.ascontiguousarray(x, dtype=np.float32)
    assert z.shape == (B, C, HZ, WZ) and x.shape == (B, C, HX, WX)

    nc = _get_nc()
    in_maps = []
    for c in range(N_CORES):
        b0 = c * B_PER_CORE
        in_maps.append({
            "xs": x[b0:b0 + B_PER_CORE].reshape(PAIRS, XF),
            "zs": z[b0:b0 + B_PER_CORE].reshape(PAIRS, ZF),
        })
    res = run_bass_kernel_spmd(nc, in_maps, list(range(N_CORES)), trace=_trace)
    out = np.empty((B, C, HO, WO), dtype=np.float32)
    for c in range(N_CORES):
        b0 = c * B_PER_CORE
        out[b0:b0 + B_PER_CORE] = res.results[c]["out"].reshape(
            B_PER_CORE, C, HO, WO)
    if _trace:
        return out, res
    return out


# revision 7
# speedup vs baseline: 2.1806x; 2.1806x over previous
"""Depthwise cross-correlation (SiamFC-style) Trainium2 kernel, v2.

z: [128, 256, 7, 7] templates, x: [128, 256, 31, 31] search images.
out[b,c,p,q] = sum_{i,j} z[b,c,i,j] * x[b,c,p+i,q+j]  -> [128, 256, 25, 25]

Pure data parallel over batch (16 batches/core on 8 cores); per core
4096 (b,c) pairs = 32 tiles of 128 partitions.

v2 engine plan (per tile), from microbenchmarked costs:
- DMA: x loaded CONTIGUOUS [128, 961] split across 2 queues (strided
  31x124B loads cost 7.7us vs 2x ~2.3us contiguous halves).
- ACT: re-pitch copy 31->32 cols (f32, ~1us) so fp32r moving APs have
  even strides; plus a few diag weight builds.
- PE: K_PE taps as fp32r full-width diag-matmul interleaved accumulation
  chains (measured 80ns per N=338 matmul: fp32r streams 2 cols/cycle).
- DVE: remaining taps as fused scalar_tensor_tensor MACs round-robin
  over NACC independent accumulators (independent ops pipeline ~5x
  better than a dependent chain), then merge tree + PSUM merges.
- GPSIMD: most diag weight builds (SBUF-only engine, otherwise idle).
"""

import numpy as np

B, C = 128, 256
HZ, WZ = 7, 7
HX, WX = 31, 31
HO, WO = 25, 25
N_CORES = 8
B_PER_CORE = B // N_CORES            # 16
PAIRS = B_PER_CORE * C               # 4096 channel pairs per core
NTILES = PAIRS // 128                # 32
XF = HX * WX                         # 961
ZF = HZ * WZ                         # 49
OF = HO * WO                         # 625
WQ = 32                              # q-window = x row pitch (flat geometry)
WXP = 32                             # padded x row pitch
XROWS = 32                           # x_t rows incl zeroed row 31
P_SPLIT = 13                         # psum chunk A rows; B = 12
NA = P_SPLIT * WQ                    # 416
NB = (HO - P_SPLIT) * WQ             # 384
NFLAT = HO * WQ                      # 800: flat window length
OHALF = 320                          # out DMA split (col units of 625)

K_PE = 29                            # taps on PE (diag matmul chains)
NACC = 4                             # DVE accumulators (indep pipelining)
XSPLIT = 544                         # x DMA split point (2 queues)


def _install_tilefix():
    """This walrus build accepts only one sync-wait command on a Drain.
    Split the TileContext tail-drain waits across single-wait SP nops."""
    import concourse.tile as tile_mod
    from concourse.vector_clock import ScopedClock

    def _drain_and_barrier_split(self, tick_clock, wait_clock):
        nc = self.nc
        probe = nc.sync.nop(nofuse=True, hint="drain_wait_probe")
        wait_clock.add_sem_waits(
            probe.ins, ScopedClock({None: tick_clock.global_clock})
        )
        si = probe.ins.sync_info
        waits = list(si.on_wait) if si is not None and si.on_wait else []
        if si is not None:
            si.on_wait = waits[:1]
        for w in waits[1:]:
            stub = nc.sync.nop(nofuse=True, hint="drain_wait_split")
            ssi = stub.ins.sync_info
            if ssi is None:
                import concourse.mybir as mybir
                stub.ins.sync_info = mybir.SyncInfo(on_wait=[w], on_update=[])
            else:
                ssi.on_wait = list(ssi.on_wait or []) + [w]
        nc.sync.drain()
        nc.all_engine_barrier()
        assert self.sems is not None
        popped = nc._tile_sem_poison_stack.pop()
        assert popped is self._sem_poison
        nc.clear_and_free_semaphores(list(self.sems.allocated().values()))
        nc.all_engine_barrier()

    tile_mod.TileContext._drain_and_barrier = _drain_and_barrier_split


def _split_multi_waits(nc):
    """This walrus build accepts only one sync-wait command per instruction.
    Hoist extra waits onto single-wait nops on the same engine just before."""
    import concourse.mybir as mybir

    n = 0
    for f in nc.m.functions:
        for bb in f.blocks:
            insts = list(bb.instructions)
            out_insts = []
            changed = False
            for inst in insts:
                si = inst.sync_info
                if si is not None and si.on_wait and len(si.on_wait) > 1:
                    waits = list(si.on_wait)
                    si.on_wait = waits[-1:]
                    for w in waits[:-1]:
                        n += 1
                        out_insts.append(mybir.InstNoOp(
                            name=f"waitsplit-{n}",
                            engine=inst.engine,
                            bass_nofuse=True,
                            sync_info=mybir.SyncInfo(on_wait=[w], on_update=[]),
                        ))
                    changed = True
                out_insts.append(inst)
            if changed:
                bb.instructions.clear()
                for inst in out_insts:
                    bb.add_instruction(inst)
    return n


_NC_CACHE = {}


def _build_bass(reps: int = 1, timing: bool = False):
    import concourse.bass as bass
    import concourse.mybir as mybir
    import concourse.tile as tile
    from concourse.masks import make_identity

    _install_tilefix()

    f32 = mybir.dt.float32
    f32r = mybir.dt.float32r

    nc = bass.Bass()
    xs = nc.declare_dram_parameter("xs", [PAIRS, XF], f32, isOutput=False)
    zs = nc.declare_dram_parameter("zs", [PAIRS, ZF], f32, isOutput=False)
    out_rows = 128 if timing else PAIRS
    out = nc.declare_dram_parameter("out", [out_rows, OF], f32, isOutput=True)

    pe_taps = list(range(K_PE))
    dve_taps = list(range(K_PE, ZF))

    with tile.TileContext(nc) as tc:
        with (
            tc.tile_pool(name="consts", bufs=1) as consts,
            tc.tile_pool(name="xcin", bufs=3) as xcin,
            tc.tile_pool(name="xin", bufs=3) as xin,
            tc.tile_pool(name="zin", bufs=3) as zin,
            tc.tile_pool(name="wts", bufs=2) as wts,
            tc.tile_pool(name="accp", bufs=2) as accp,
            tc.tile_pool(name="outp", bufs=3) as outp,
            tc.tile_pool(name="psum", bufs=3, space="PSUM") as psum,
        ):
            ident = consts.tile([128, 128], f32)
            make_identity(nc, ident)
            ident_r = consts.tile([128, 128], f32r)
            nc.vector.tensor_scalar_mul(ident_r, ident, 1.0)
            id1 = consts.tile([128, 1, 128], f32)
            nc.vector.tensor_copy(id1, ident)

            for _rep in range(reps):
              for t in range(NTILES):
                r0 = t * 128
                # contiguous x load, split across two DMA queues; zero tail
                xc = xcin.tile([128, 1024], f32)
                nc.sync.dma_start(out=xc[:, 0:XSPLIT],
                                  in_=xs[r0:r0 + 128, 0:XSPLIT])
                nc.gpsimd.dma_start(out=xc[:, XSPLIT:XF],
                                    in_=xs[r0:r0 + 128, XSPLIT:XF])
                nc.vector.memset(xc[:, XF:1024], 0.0)
                z_t = zin.tile([128, ZF], f32)
                nc.sync.dma_start(out=z_t, in_=zs[r0:r0 + 128, :])

                # x_t: pitch-32 fp32r copy of x; pad col/rows zeroed
                x_t = xin.tile([128, XROWS, WXP], f32r)
                x_flat = x_t.rearrange("p h w -> p (h w)")
                x_f = x_t.bitcast(f32)
                xf_flat = x_f.rearrange("p h w -> p (h w)")
                nc.vector.tensor_scalar_mul(x_flat, xc, 0.0)
                nc.scalar.copy(
                    x_t[:, 0:HX, 0:WX],
                    xc[:, 0:XF].rearrange("p (h w) -> p h w", h=HX))

                # one-shot diag weight build for PE taps (broadcast AP)
                w_all = wts.tile([128, K_PE, 128], f32r)
                nc.vector.tensor_tensor(
                    w_all,
                    id1[:, :, :].broadcast_to([128, K_PE, 128]),
                    z_t[:, 0:K_PE].broadcast_to([128, K_PE, 128]),
                    op=mybir.AluOpType.mult)

                # PE: interleaved two-bank fp32r accumulation chains
                ps_a = psum.tile([128, P_SPLIT, WQ], f32)
                ps_b = psum.tile([128, HO - P_SPLIT, WQ], f32)

                def winp(i, j, p0, pc):
                    off = WXP * (i + p0) + j
                    return x_flat[:, off:off + pc * WXP].rearrange(
                        "p (a b) -> p a b", a=pc)

                for n in range(K_PE):
                    i, j = divmod(n, WZ)
                    w_n = w_all[:, n, :]
                    nc.tensor.matmul(
                        ps_a, w_n, winp(i, j, 0, P_SPLIT),
                        start=(n == 0), stop=False,
                        skip_group_check=True)
                    nc.tensor.matmul(
                        ps_b, w_n, winp(i, j, P_SPLIT, HO - P_SPLIT),
                        start=(n == 0), stop=False,
                        skip_group_check=True)

                # DVE: fused flat MAC taps round-robin over NACC accs
                acc = accp.tile([128, NACC, NFLAT], f32)
                for n, tap in enumerate(range(K_PE, ZF)):
                    i, j = divmod(tap, WZ)
                    k = n % NACC
                    x_win = xf_flat[:, WXP * i + j:WXP * i + j + NFLAT]
                    zcol = z_t[:, tap:tap + 1]
                    if n < NACC:
                        nc.vector.tensor_scalar_mul(acc[:, k], x_win, zcol)
                    else:
                        nc.vector.scalar_tensor_tensor(
                            acc[:, k], x_win, zcol, acc[:, k],
                            op0=mybir.AluOpType.mult,
                            op1=mybir.AluOpType.add)

                # merge accs (flat adds), then add into PSUM via PE ident
                m_t = accp.tile([128, NFLAT], f32r, name="m_t")
                m_f = m_t.bitcast(f32)
                nc.vector.tensor_add(acc[:, 0], acc[:, 0], acc[:, 1])
                nc.vector.tensor_add(acc[:, 2], acc[:, 2], acc[:, 3])
                nc.vector.tensor_tensor(
                    m_t, acc[:, 0], acc[:, 2], op=mybir.AluOpType.add)
                nc.tensor.matmul(
                    ps_a, ident_r,
                    m_t[:, 0:NA].rearrange("p (a b) -> p a b", a=P_SPLIT),
                    start=False, stop=True, skip_group_check=True)
                nc.tensor.matmul(
                    ps_b, ident_r,
                    m_t[:, NA:NFLAT].rearrange("p (a b) -> p a b",
                                               a=HO - P_SPLIT),
                    start=False, stop=True, skip_group_check=True)

                # ACT: compact psum (pitch 32 -> 25) into output staging
                o_t = outp.tile([128, HO, WO], f32)
                nc.scalar.copy(o_t[:, 0:P_SPLIT, :], ps_a[:, :, 0:WO])
                nc.scalar.copy(o_t[:, P_SPLIT:HO, :], ps_b[:, :, 0:WO])

                o0 = 0 if timing else r0
                o_flat = o_t.rearrange("p h w -> p (h w)")
                nc.scalar.dma_start(out=out[o0:o0 + 128, 0:OHALF],
                                    in_=o_flat[:, 0:OHALF])
                nc.gpsimd.dma_start(out=out[o0:o0 + 128, OHALF:OF],
                                     in_=o_flat[:, OHALF:OF])

    _split_multi_waits(nc)
    return nc


def _get_nc(reps: int = 1, timing: bool = False):
    key = ("nc", reps, timing)
    if key not in _NC_CACHE:
        _NC_CACHE[key] = _build_bass(reps, timing)
    return _NC_CACHE[key]


def kernel(z: np.ndarray, x: np.ndarray, _trace: bool = False):
    from concourse.bass_utils import run_bass_kernel_spmd

    z = np.ascontiguousarray(z, dtype=np.float32)
    x = np.ascontiguousarray(x, dtype=np.float32)
    assert z.shape == (B, C, HZ, WZ) and x.shape == (B, C, HX, WX)

    nc = _get_nc()
    in_maps = []
    for c in range(N_CORES):
        b0 = c * B_PER_CORE
        in_maps.append({
            "xs": x[b0:b0 + B_PER_CORE].reshape(PAIRS, XF),
            "zs": z[b0:b0 + B_PER_CORE].reshape(PAIRS, ZF),
        })
    res = run_bass_kernel_spmd(nc, in_maps, list(range(N_CORES)), trace=_trace)
    out = np.empty((B, C, HO, WO), dtype=np.float32)
    for c in range(N_CORES):
        b0 = c * B_PER_CORE
        out[b0:b0 + B_PER_CORE] = res.results[c]["out"].reshape(
            B_PER_CORE, C, HO, WO)
    if _trace:
        return out, res
    return out
